# revision 30
# baseline (speedup 1.0000x reference)
"""Trainium2 Bass kernel for CorrelatedSphericalField sampling (bf16 v5).

Math (validated against the jax reference):
  coeffs[t] = PHI^t * d_t,   d_t = d_{t-1} + PHI^{-t} * sigma_n (.) xi_{t-1},  d_0 = coeff0
  xs[t,n,k,m] = sum_l d[t,n,l,m] * pct[m,l,k]          (per-m Legendre GEMM)
  out[t,n,k,j] = 4pi * PHI^t * irfft_j(xs), as half-spectrum GEMMs:
      A[.., j] = sum_m xs_re[.., m] C[m, j],  B[.., j] = sum_m xs_im[.., m] S[m, j]
      out[.., 0:362] = A + B ;  out[.., 362+jj] = (A - B)[.., 360-jj]
  PHI^t and 4pi are folded into per-core C/S constants.

Distribution (8 cores, single launch):
  stages A+B sharded over m (46 of 368 zero-padded m's per core, all (t,n)),
  processed in 3 m-groups (16/16/14) pipelined with a chunked AllToAll of
  xs (shard dim = t); stage D sharded over t (core c handles t=c).

v5 changes vs v4: m-major A2A blocks (contiguous recv loads), exact-layout
pct pair loads, 3 A2A groups, stage D reads PSUM directly (no scalar
copies), bf16 output (host converts to f32).
"""
import numpy as np
import ml_dtypes

import concourse.bass as bass
import concourse.mybir as mybir
import concourse.tile as tile
from concourse.bass_utils import run_bass_kernel_spmd

# ---- problem constants (hardcoded; kernel must be self-contained) ----
T = 8
N = 16
L = 361          # number of degrees l (contraction dim of stage B)
L2 = 384         # L zero-padded to 3*128
KLAT = 361       # number of latitudes
M = 362          # number of orders m
NLON = 722
JH = 362         # half-spectrum output columns of stage D
NC = 8
MPAD = 368       # M padded to a multiple of NC
MC = MPAD // NC  # 46 m's per core
TN = T * N       # 128
E = 2
MEN = MC * E * N  # 1472

PHI = float(np.exp(-6.0 / 48.0))
FOUR_PI = float(4.0 * np.pi)

LCH = [(0, 128), (128, 256), (256, 384)]
KCH = [(0, 128), (128, 256), (256, 361)]
# A2A m-chunks within a core (pair-aligned group boundaries)
MGRP = [(0, 16), (16, 32), (32, 46)]
G = len(MGRP)
# global m is reindexed group-major: m' = (g, core, local); stage-D m-chunks
# then coincide with A2A groups (sizes 128/128/112)
GSZ = [(gb - ga) * NC for (ga, gb) in MGRP]
# A2A block layout: True = m-major [E, mg, N, K] blocks (contiguous recv
# loads); False = v4-style tn-major [N, E, mg, K] blocks (transposed recvs)
# The collective tensors are always declared flat [T, E*mg*N*K]; the
# write/read DMAs use rearranged views.
# NOTE: M_MAJOR=True is BROKEN at runtime — the (t n) partition-split SBUF
# source AP silently mislowers in the DMA engine (garbage results).
M_MAJOR = False
# sigma' packed offsets: per group block [T, m_g, e] at SIG_OFF[g]
SIG_OFF = [T * E * ga for (ga, gb) in MGRP]
# xi packed offsets: per group block [t, m_g, e, n] at XI_OFF[g]
XI_OFF = [T * E * N * ga for (ga, gb) in MGRP]

F32 = mybir.dt.float32
BF16 = mybir.dt.bfloat16
NPBF = ml_dtypes.bfloat16


def _split_multi_waits(nc, max_inline=1):
    """The walrus build in this env accepts only one inline sync-wait per
    instruction; hoist extras onto same-engine NoOps placed just before."""
    ctr = 0
    for f in nc.m.functions:
        for bb in f.blocks:
            new = []
            for inst in bb.instructions:
                si = inst.sync_info
                if si is not None and si.on_wait and len(si.on_wait) > max_inline:
                    waits = list(si.on_wait)
                    keep = waits[-max_inline:]
                    for w in waits[:-max_inline]:
                        ctr += 1
                        nop = mybir.InstNoOp(name=f"I-wsplit-{ctr}",
                                             engine=inst.engine)
                        nop.sync_info = mybir.SyncInfo(on_wait=[w], on_update=[])
                        new.append(nop)
                    inst.sync_info = mybir.SyncInfo(
                        on_wait=keep, on_update=list(si.on_update))
                new.append(inst)
            bb.instructions = new


def build_nc(split_waits=True):
    nc = bass.Bass(num_devices=NC)

    # host layouts: xi [l, group-packed (t, m_g, e, n)], c0 [l, (m, e, n)],
    # sigp [l, group-packed (t, m_g, e)], pct pairs [MC//2, 128, 2, 3, k]
    xi_p = nc.declare_dram_parameter("xi_t", [L2, T * MC * E * N], BF16,
                                     isOutput=False)
    c0_p = nc.declare_dram_parameter("c0_t", [L2, MEN], BF16, isOutput=False)
    sig_p = nc.declare_dram_parameter("sigp", [L2, T * MC * E], BF16, isOutput=False)
    pct_p = nc.declare_dram_parameter("pct_t", [MC // 2, 128, 2, 3, KLAT], BF16,
                                      isOutput=False)
    csC_p = nc.declare_dram_parameter("csC", [MPAD, JH], BF16, isOutput=False)
    csS_p = nc.declare_dram_parameter("csS", [MPAD, JH], BF16, isOutput=False)
    # device emits S=A+B | D=A-B straight; host mirrors D into columns JH..NLON
    out_p = nc.declare_dram_parameter("out_t", [N, KLAT, 2 * JH], BF16,
                                      isOutput=True)

    with tile.TileContext(nc) as tc:
        with tc.tile_pool(name="dram", bufs=1, space="DRAM") as pdram:
            sends, recvs = [], []
            for g, (ga, gb) in enumerate(MGRP):
                mg = gb - ga
                if M_MAJOR:
                    # block t -> flat [E * mg * N * KLAT]
                    shp = [T, E * mg * N * KLAT]
                else:
                    # v4 layout: block t -> [N, E, mg, KLAT] (dim0 = (t n))
                    shp = [TN, E, mg, KLAT]
                sends.append(pdram.tile(shp, BF16,
                                        name=f"send{g}", tag=f"send{g}"))
                recvs.append(pdram.tile(shp, BF16,
                                        name=f"recv{g}", tag=f"recv{g}"))

            with (
                tc.tile_pool(name="per", bufs=1) as pa,
                tc.tile_pool(name="cs", bufs=1) as pcs,
                tc.tile_pool(name="xr", bufs=1) as pxr,
                tc.tile_pool(name="xi", bufs=2) as px,
                tc.tile_pool(name="w", bufs=6) as pw,
                tc.tile_pool(name="xs", bufs=6) as pxs,
                tc.tile_pool(name="psB", bufs=3, space="PSUM") as pp,
            ):
                # stage-D constants loaded up front (group-major row order)
                csC_t, csS_t = [], []
                goff = 0
                for g in range(G):
                    ct = pcs.tile([GSZ[g], JH], BF16, name=f"csC{g}", tag=f"csC{g}")
                    st = pcs.tile([GSZ[g], JH], BF16, name=f"csS{g}", tag=f"csS{g}")
                    nc.sync.dma_start(ct[:], csC_p[goff:goff + GSZ[g]])
                    nc.sync.dma_start(st[:], csS_p[goff:goff + GSZ[g]])
                    csC_t.append(ct)
                    csS_t.append(st)
                    goff += GSZ[g]
                xr = {}
                for e in range(E):
                    for g in range(G):
                        xr[(e, g)] = pxr.tile([GSZ[g], N * KLAT], BF16,
                                              name=f"xr{e}{g}", tag=f"xr{e}{g}")

                # persistent: sigma', per-(lc,g) d tiles
                sig_tiles = []
                for lc, (la, lb) in enumerate(LCH):
                    st_ = pa.tile([128, T * MC * E], BF16, tag=f"sig{lc}")
                    nc.sync.dma_start(st_[:], sig_p[la:lb])
                    sig_tiles.append(st_)
                d_tiles = {}
                for g, (ga, gb) in enumerate(MGRP):
                    me_g = (gb - ga) * E
                    for lc in range(3):
                        d_tiles[(lc, g)] = pa.tile([128, me_g, T, N], BF16,
                                                   name=f"d{lc}g{g}",
                                                   tag=f"d{lc}g{g}")

                for g, (ga, gb) in enumerate(MGRP):
                    sz = gb - ga
                    me_g = sz * E
                    men_g = me_g * N
                    # ---- stage A for group g -------------------------------
                    for lc, (la, lb) in enumerate(LCH):
                        dt_ = d_tiles[(lc, g)]
                        xi_sb = px.tile([128, T * men_g], BF16, tag="xi")
                        c0_sb = px.tile([128, men_g], BF16, tag="c0s")
                        nc.sync.dma_start(
                            xi_sb[:],
                            xi_p[la:lb, XI_OFF[g]:XI_OFF[g] + T * men_g])
                        nc.sync.dma_start(
                            c0_sb[:], c0_p[la:lb, ga * E * N:gb * E * N])
                        sig_b = sig_tiles[lc][
                            :, SIG_OFF[g]:SIG_OFF[g] + T * me_g][
                            :, :, None].broadcast_to([128, T * me_g, N])
                        # z = sigma' (.) xi, computed in place in xi_sb
                        z_v = xi_sb[:].rearrange("p (tq n) -> p tq n", n=N)
                        nc.vector.tensor_tensor(
                            out=z_v, in0=z_v, in1=sig_b,
                            op=mybir.AluOpType.mult)
                        nc.vector.tensor_copy(
                            dt_[:, :, 0, :],
                            c0_sb[:].rearrange("p (q n) -> p q n", n=N))
                        for t in range(1, T):
                            nc.vector.tensor_tensor(
                                out=dt_[:, :, t, :],
                                in0=dt_[:, :, t - 1, :],
                                in1=z_v[:, (t - 1) * me_g:t * me_g, :],
                                op=mybir.AluOpType.add)

                    # ---- stage B for group g -------------------------------
                    for pr in range(ga // 2, gb // 2):
                        w = pw.tile([128, 2, 3, KLAT], BF16, tag="pct")
                        nc.sync.dma_start(w[:], pct_p[pr])
                        for mi in range(2):
                            m = 2 * pr + mi
                            gm = m - ga
                            xs_sb = pxs.tile([TN, E, KLAT], BF16, tag="xsb")
                            for e in range(E):
                                ps = pp.tile([TN, KLAT], F32, tag=f"ps{e}")
                                for lc in range(3):
                                    nc.tensor.matmul(
                                        ps[:],
                                        d_tiles[(lc, g)][:, gm * E + e],
                                        w[:, mi, lc],
                                        start=(lc == 0), stop=(lc == 2))
                                # PSUM -> SBUF bf16
                                if e == 0:
                                    nc.scalar.copy(xs_sb[:, 0], ps[:])
                                else:
                                    nc.vector.tensor_copy(xs_sb[:, 1], ps[:])
                                if M_MAJOR:
                                    dst = sends[g].rearrange(
                                        "t (e m n k) -> t e m n k",
                                        e=E, m=sz, n=N)[:, e, gm]
                                    nc.scalar.dma_start(
                                        dst,
                                        xs_sb[:, e].rearrange(
                                            "(t n) k -> t n k", t=T))
                            if not M_MAJOR:
                                nc.scalar.dma_start(
                                    sends[g][:, :, gm], xs_sb[:])

                    nc.gpsimd.collective_compute(
                        "AllToAll", mybir.AluOpType.bypass,
                        replica_groups=[list(range(NC))],
                        ins=[sends[g].opt()], outs=[recvs[g].opt()])

                    # xs-recv gathers for this group, issued from the gpsimd
                    # queue right after its collective: they fire as soon as
                    # the A2A lands, prefetching xr so stage-D partial chains
                    # can run during the NEXT group's A2A.
                    mg = gb - ga
                    for e in range(E):
                        for c in range(NC):
                            nc.gpsimd.dma_start(
                                xr[(e, g)][c * mg:(c + 1) * mg].rearrange(
                                    "p (n k) -> p n k", n=N),
                                recvs[g][16 * c:16 * (c + 1), e]
                                .transpose([1, 0, 2]))

            # ---------------- stage D: iFFT GEMM over m ---------------------
            with (
                tc.tile_pool(name="o", bufs=6) as po,
                tc.tile_pool(name="psD", bufs=4, space="PSUM") as pp2,
            ):
                for n in range(N):
                    for (ka, kb) in KCH:
                        kp = kb - ka
                        psA = pp2.tile([kp, JH], F32, tag="psA")
                        psB = pp2.tile([kp, JH], F32, tag="psB")
                        for g in range(G):
                            nc.tensor.matmul(
                                psA[:],
                                xr[(0, g)][:, n * KLAT + ka:n * KLAT + kb],
                                csC_t[g][:],
                                start=(g == 0), stop=(g == G - 1))
                        for g in range(G):
                            nc.tensor.matmul(
                                psB[:],
                                xr[(1, g)][:, n * KLAT + ka:n * KLAT + kb],
                                csS_t[g][:],
                                start=(g == 0), stop=(g == G - 1))
                        # PSUM has a single DVE read port: stage psB in SBUF,
                        # then S/D with one PSUM operand each (no reversal on
                        # device; host mirrors D)
                        b_sb = po.tile([kp, JH], F32, tag="b_sb")
                        oo = po.tile([kp, 2 * JH], BF16, tag="oo")
                        nc.scalar.copy(b_sb[:], psB[:])
                        nc.vector.tensor_tensor(
                            out=oo[:, 0:JH], in0=psA[:], in1=b_sb[:],
                            op=mybir.AluOpType.add)
                        nc.vector.tensor_tensor(
                            out=oo[:, JH:2 * JH], in0=psA[:], in1=b_sb[:],
                            op=mybir.AluOpType.subtract)
                        nc.sync.dma_start(out_p[n, ka:kb], oo[:])

    if split_waits:
        _split_multi_waits(nc)
    return nc


def prep_inputs(x, sigma_n, coeff0, xi, pct):
    """Host-side shard/stage: slice + transpose per-core inputs, build constants."""
    sigma_n = np.asarray(sigma_n, np.float32)
    coeff0 = np.asarray(coeff0, np.float32)
    xi = np.asarray(xi, np.float32)
    pct = np.asarray(pct, np.float32)

    padm = MPAD - M
    padl = L2 - L
    sig_pad = np.pad(sigma_n, ((0, padl), (0, padm)))
    c0_pad = np.pad(coeff0, ((0, 0), (0, padl), (0, padm), (0, 0)))
    xi_pad = np.pad(xi, ((0, 0), (0, 0), (0, padl), (0, padm), (0, 0)))
    pct_pad = np.pad(pct, ((0, padm), (0, padl), (0, 0)))

    # half-spectrum irfft matrices (fp64 host build)
    j = np.arange(JH, dtype=np.float64)
    mm = np.arange(M, dtype=np.float64)
    ang = 2.0 * np.pi * np.outer(mm, j) / NLON
    Cm = 2.0 * np.cos(ang)
    Cm[0, :] = 1.0
    Cm[M - 1, :] = np.cos(np.pi * j)
    Sm = -2.0 * np.sin(ang)
    Sm[0, :] = 0.0
    Sm[M - 1, :] = 0.0
    Cp = np.pad(Cm, ((0, padm), (0, 0)))
    Sp = np.pad(Sm, ((0, padm), (0, 0)))
    # group-major row order: m' = (g, core, local-in-group)
    perm = np.array([c * MC + i for (ga, gb) in MGRP
                     for c in range(NC) for i in range(ga, gb)])
    Cp = Cp[perm]
    Sp = Sp[perm]

    phi_inv = PHI ** -(np.arange(T, dtype=np.float64) + 1.0)

    in_maps = []
    for c in range(NC):
        msl = slice(c * MC, (c + 1) * MC)
        # xi group-major: per group block [l, t, m_g, e, n]
        xi_c = np.transpose(xi_pad[:, :, :, msl, :], (2, 0, 3, 4, 1))  # l,t,m,e,n
        xi_blocks = [np.ascontiguousarray(
            xi_c[:, :, ga:gb]).reshape(L2, -1) for (ga, gb) in MGRP]
        xi_g = np.concatenate(xi_blocks, axis=1)
        # [n,l,m,e] -> [l,m,e,n]
        c0_c = np.ascontiguousarray(
            np.transpose(c0_pad[:, :, msl, :], (1, 2, 3, 0))
        ).reshape(L2, MEN).astype(NPBF)
        # sigma' group-packed: per group block [t, m_g, e]
        sig_me = (sig_pad[:, None, msl] * phi_inv[None, :, None])  # [l, t, m]
        sig_me = np.repeat(sig_me[:, :, :, None], E, axis=3)       # [l, t, m, e]
        blocks = [np.ascontiguousarray(sig_me[:, :, ga:gb]).reshape(L2, -1)
                  for (ga, gb) in MGRP]
        sig_c = np.concatenate(blocks, axis=1)
        # pct pairs: [pr, p, mi, lc, k] with l = lc*128 + p, m = 2*pr + mi
        pct_c = pct_pad[msl]                                       # [46, 384, 361]
        pct_pr = np.transpose(
            pct_c.reshape(MC // 2, 2, 3, 128, KLAT), (0, 3, 1, 2, 4))
        scale = FOUR_PI * PHI ** c
        in_maps.append({
            "xi_t": np.ascontiguousarray(xi_g).astype(NPBF),
            "c0_t": c0_c,
            "sigp": np.ascontiguousarray(sig_c).astype(NPBF),
            "pct_t": np.ascontiguousarray(pct_pr).astype(NPBF),
            "csC": (scale * Cp).astype(NPBF),
            "csS": (scale * Sp).astype(NPBF),
        })
    return in_maps


_NC_CACHE = None


def kernel(x, sigma_n, coeff0, xi, pct):
    global _NC_CACHE
    in_maps = prep_inputs(x, sigma_n, coeff0, xi, pct)
    if _NC_CACHE is None:
        _NC_CACHE = build_nc()
    res = run_bass_kernel_spmd(_NC_CACHE, in_maps, list(range(NC)))
    return assemble_out(res)


def assemble_out(res):
    """Stack per-core S|D halves, mirror D into the upper lon columns."""
    sd = np.stack([np.asarray(res.results[c]["out_t"], dtype=np.float32)
                   for c in range(NC)], axis=0)       # [T, N, KLAT, 2*JH]
    out = np.empty((T, N, KLAT, NLON), dtype=np.float32)
    out[..., :JH] = sd[..., :JH]
    out[..., JH:] = sd[..., JH + 1:2 * JH - 1][..., ::-1]
    return out.reshape(T, 1, 1, N, KLAT, NLON)
